# revision 26
# baseline (speedup 1.0000x reference)
"""Blockwise-dropout GEMM (DropoutMM) for 8x Trainium2 NeuronCores — v3.

out = (x * expand(block_mask) / (1-p)) @ weight.T
  x: [8192, 4096] f32, weight: [4096, 4096] f32, block_mask: [64, 32] i32

v3 strategy (on top of v2's 2-group M-split x 4-way N-split, trace-time
block skipping, fp16 matmuls). Measured ~212-215us vs v2's ~245us; rel err
1.956e-2 (deterministic, FP8_FRAC=0.375) vs the 2e-2 gate:
  - Mixed-precision schedule: FP8_FRAC of each row's kept k-blocks is
    computed as fp8e4 (e4m3) DoubleRow PAIRS — two k-blocks interleaved per
    matmul at 2x PE rate — the rest stays fp16 (fp8-everything would be
    ~3.2e-2 err; fp16-only is 2.5e-4; err scales as 3.2e-2*sqrt(frac)).
    Pairs sit at each slot's tail, spread evenly across slots:
    concentrating DoubleRow work on all 8 cores at once trips a chip-level
    GPIO power brake (~0.81 util duty cycle for the rest of the kernel).
  - Both W copies are pre-scaled by 64 (w8 = e4m3(w*scale*64) to clear
    e4m3's subnormal floor at sigma~=1/64; w16 = fp16(w*scale*64) so both
    precisions accumulate into ONE PSUM bank), and the evacuation applies
    x 1/64 — no extra combine ops.
  - Weight preload (both precisions) on the scalar HWDGE ring as ONE
    combined first-use-ordered stream, full 2KB-per-partition-line blocks
    (the gpsimd ring is a slow software-DMA path; half-blocks halve DMA
    line size and cost ~27us). First FP8_FREE_SLOTS slots stay pure-fp16.
  - N_WARM dep-free warmup matmuls keep the PE busy through the ~15us
    p-state ramp window (idle gaps reset the ramp) while weights stream.
  - Evacuation split DVE(nt0)/ACT(nt1); stores split across the ACT and SP
    rings per half; the last slot evacuates in quarters to shrink the
    final drain.
"""

import os
import sys

import numpy as np

for _p in ("/opt/trn_rl_repo", "/root/.axon_site/_ro/trn_rl_repo"):
    if os.path.isdir(_p) and _p not in sys.path:
        sys.path.insert(0, _p)

BLOCK = 128
P_DROP = 0.1
N_CORES = 8
N_GROUPS = 2
CORES_PER_GROUP = N_CORES // N_GROUPS
MM_FREE = 512  # matmul moving free dim (one PSUM bank of fp32)
W_SCALE = 64.0  # fp8 weight pre-scale (power of 2; undone at evacuation)
FP8_FRAC = float(os.environ.get("KERNEL_FP8_FRAC", "0.375"))
FP8_FREE_SLOTS = int(os.environ.get("KERNEL_FREE_SLOTS", "2"))  # first slots per group stay pure fp16 (PE ramp)
N_WARM = int(os.environ.get("KERNEL_N_WARM", "28"))

LAST_RUN_INFO = {}


def _build_program(kept16, pairs, nsh, kb_blocks):
    """One group's program: exact block-skip GEMM over len(kept16) slots.

    kept16[s]: fp16 block indices for slot s; pairs[s]: list of (b0, b1)
    fp8 DoubleRow pairs (b0 < b1). Accumulation is 64x the true output;
    evacuation multiplies by 1/64.
    """
    from concourse import bacc
    import concourse.mybir as mybir
    import concourse.tile as tile

    P = BLOCK
    mb = len(kept16)
    c16 = [len(k) for k in kept16]
    np8 = [len(p) for p in pairs]
    tot16 = int(sum(c16))
    tot8 = int(sum(np8))
    cmax16 = max(1, max(c16))
    pmax = max(1, max(np8))
    nt_tiles = nsh // MM_FREE

    nc = bacc.Bacc("TRN2", target_bir_lowering=False)
    XC = nc.dram_tensor("XC", [P, max(tot16, 1) * P], mybir.dt.float16,
                        kind="ExternalInput")
    XP = nc.dram_tensor("XP", [P, max(tot8, 1), 2, P], mybir.dt.float8e4,
                        kind="ExternalInput")
    WS = nc.dram_tensor("WS", [kb_blocks, P, nsh], mybir.dt.float16,
                        kind="ExternalInput")
    W8 = nc.dram_tensor("W8", [kb_blocks, P, nsh], mybir.dt.float8e4,
                        kind="ExternalInput")
    OUT = nc.dram_tensor("OUT", [P, mb, nsh], mybir.dt.float32,
                         kind="ExternalOutput")

    # Preload order: combined first-use list over the slot program across
    # both precisions; all emitted on the scalar HWDGE ring so delivery
    # tracks need-order.
    w_order = []  # (is_fp8, block)
    seen16, seen8 = set(), set()
    for s in range(mb):
        for b0, b1 in pairs[s]:
            for b in (int(b0), int(b1)):
                if b not in seen8:
                    seen8.add(b)
                    w_order.append((True, b))
        for b in kept16[s]:
            if int(b) not in seen16:
                seen16.add(int(b))
                w_order.append((False, int(b)))

    with tile.TileContext(nc) as tc:
        with (
            tc.tile_pool(name="wpool", bufs=1) as wpool,
            tc.tile_pool(name="xpool", bufs=4) as xpool,
            tc.tile_pool(name="opool", bufs=3) as opool,
            tc.tile_pool(name="psum", bufs=8, space="PSUM") as psum,
        ):
            # HAM warmup: dummy matmuls with no DMA deps run during the fixed
            # kernel preamble so the PE clock is ramped when real work starts.
            wa = xpool.tile([P, P], mybir.dt.bfloat16, tag="warm_a", bufs=1)
            wb = xpool.tile([P, MM_FREE], mybir.dt.bfloat16, tag="warm_b", bufs=1)
            nc.gpsimd.memset(wa, 0.0)
            nc.gpsimd.memset(wb, 0.0)
            wp = psum.tile([P, MM_FREE], mybir.dt.float32, tag="ps", name="warm_ps")
            for _ in range(N_WARM):
                nc.tensor.matmul(wp, wa, wb, start=True, stop=True)

            # All weight preloads on the scalar HWDGE ring in need order —
            # the gpsimd ring is a SOFTWARE dma path (slow descriptor
            # generation) and starves the PE.
            w_res = wpool.tile([P, kb_blocks, nsh], mybir.dt.float16, tag="w")
            w8_res = wpool.tile([P, kb_blocks, nsh], mybir.dt.float8e4, tag="w8")
            for i, (is8, b) in enumerate(w_order):
                eng = nc.scalar if i % 2 == 0 else nc.sync
                if is8:
                    eng.dma_start(out=w8_res[:, b], in_=W8[b])
                else:
                    eng.dma_start(out=w_res[:, b], in_=WS[b])

            off16 = 0
            off8 = 0
            for s in range(mb):
                c = c16[s]
                p = np8[s]
                ot = opool.tile([P, nsh], mybir.dt.float32, tag="o")
                if c == 0 and p == 0:
                    nc.any.memset(ot, 0.0)
                    nc.scalar.dma_start(out=OUT[:, s, :], in_=ot[:])
                    continue
                if c > 0:
                    xt = xpool.tile(
                        [P, cmax16 * P], mybir.dt.float16, tag="x", name=f"x_{s}"
                    )
                    nc.sync.dma_start(
                        out=xt[:, : c * P], in_=XC[:, off16 * P : (off16 + c) * P]
                    )
                if p > 0:
                    xt8 = xpool.tile(
                        [P, pmax, 2, P], mybir.dt.float8e4, tag="x8", name=f"x8_{s}"
                    )
                    nc.sync.dma_start(
                        out=xt8[:, :p], in_=XP[:, off8 : off8 + p]
                    )
                pts = [
                    psum.tile(
                        [P, MM_FREE], mybir.dt.float32, tag="ps", name=f"ps_{s}_{nt}"
                    )
                    for nt in range(nt_tiles)
                ]
                units = p + c
                u = 0
                # fp8 pairs first, then fp16 singles; j-outer / nt-inner so
                # consecutive matmuls share the stationary operand.
                for j in range(p):
                    b0, b1 = int(pairs[s][j][0]), int(pairs[s][j][1])
                    st = b1 - b0
                    for nt in range(nt_tiles):
                        nc.tensor.matmul(
                            pts[nt],
                            xt8[:, j],
                            w8_res[:, b0 : b1 + 1 : st,
                                   nt * MM_FREE : (nt + 1) * MM_FREE],
                            start=(u == 0),
                            stop=(u == units - 1),
                            perf_mode=mybir.MatmulPerfMode.DoubleRow,
                        )
                    u += 1
                for j in range(c):
                    b = int(kept16[s][j])
                    for nt in range(nt_tiles):
                        nc.tensor.matmul(
                            pts[nt],
                            xt[:, j * P : (j + 1) * P],
                            w_res[:, b, nt * MM_FREE : (nt + 1) * MM_FREE],
                            start=(u == 0),
                            stop=(u == units - 1),
                        )
                    u += 1
                # Evacuation split across DVE (nt0) and ACT (nt1); stores
                # split per half across the ACT and SP rings — halves the
                # per-slot evacuation latency and the end-of-kernel flush.
                # The LAST slot goes in quarters for a shorter final drain.
                if s == mb - 1:
                    Q = MM_FREE // 2
                    for q in range(4):
                        sl = slice(q * Q, (q + 1) * Q)
                        psl = slice((q % 2) * Q, (q % 2 + 1) * Q)
                        eng = nc.vector if q % 2 == 0 else None
                        if eng is not None:
                            eng.tensor_scalar_mul(
                                out=ot[:, sl], in0=pts[q // 2][:, psl],
                                scalar1=1.0 / W_SCALE,
                            )
                        else:
                            nc.scalar.activation(
                                out=ot[:, sl],
                                in_=pts[q // 2][:, psl],
                                func=mybir.ActivationFunctionType.Copy,
                                scale=1.0 / W_SCALE,
                            )
                        ring = nc.scalar if q % 2 == 0 else nc.sync
                        ring.dma_start(out=OUT[:, s, sl], in_=ot[:, sl])
                else:
                    nc.vector.tensor_scalar_mul(
                        out=ot[:, 0:MM_FREE], in0=pts[0], scalar1=1.0 / W_SCALE
                    )
                    nc.scalar.activation(
                        out=ot[:, MM_FREE : 2 * MM_FREE],
                        in_=pts[1],
                        func=mybir.ActivationFunctionType.Copy,
                        scale=1.0 / W_SCALE,
                    )
                    nc.scalar.dma_start(
                        out=OUT[:, s, 0:MM_FREE], in_=ot[:, 0:MM_FREE]
                    )
                    nc.sync.dma_start(
                        out=OUT[:, s, MM_FREE : 2 * MM_FREE],
                        in_=ot[:, MM_FREE : 2 * MM_FREE],
                    )
                off16 += c
                off8 += p
    nc.compile()
    return nc


def _make_fn(nc, devices):
    """Replicates bass2jax.run_bass_via_pjrt's multi-core path for an
    arbitrary device subset; returns an async-dispatchable jitted fn."""
    import jax
    import concourse.mybir as mybir
    from concourse.bass2jax import (
        _bass_exec_p,
        install_neuronx_cc_hook,
        partition_id_tensor,
    )
    from jax.experimental.shard_map import shard_map
    from jax.sharding import Mesh, PartitionSpec

    install_neuronx_cc_hook()

    partition_name = nc.partition_id_tensor.name if nc.partition_id_tensor else None
    in_names, out_names, out_avals = [], [], []
    for alloc in nc.m.functions[0].allocations:
        if not isinstance(alloc, mybir.MemoryLocationSet):
            continue
        name = alloc.memorylocations[0].name
        if alloc.kind == "ExternalInput":
            if name != partition_name:
                in_names.append(name)
        elif alloc.kind == "ExternalOutput":
            shape = tuple(alloc.tensor_shape)
            dtype = mybir.dt.np(alloc.dtype)
            out_names.append(name)
            out_avals.append(jax.core.ShapedArray(shape, dtype))
    n_params = len(in_names)
    all_names = list(in_names) + list(out_names)
    if partition_name is not None:
        all_names.append(partition_name)

    def _body(*args):
        operands = list(args)
        if partition_name is not None:
            operands.append(partition_id_tensor())
        outs = _bass_exec_p.bind(
            *operands,
            out_avals=tuple(out_avals),
            in_names=tuple(all_names),
            out_names=tuple(out_names),
            lowering_input_output_aliases=(),
            sim_require_finite=True,
            sim_require_nnan=True,
            nc=nc,
        )
        return tuple(outs)

    mesh = Mesh(np.asarray(devices), ("core",))
    n_outs = len(out_names)
    donate = tuple(range(n_params, n_params + n_outs))
    fn = jax.jit(
        shard_map(
            _body,
            mesh=mesh,
            in_specs=(PartitionSpec("core"),) * (n_params + n_outs),
            out_specs=(PartitionSpec("core"),) * n_outs,
            check_rep=False,
        ),
        donate_argnums=donate,
        keep_unused=True,
    )
    return fn, in_names, out_names, out_avals, mesh


def _host_prep_group(x4, rows, kept16, pairs, mask_vals=None):
    """XC (fp16 singles) and XP (fp8 pairs) for one group.

    XC: [128, tot16*128] fp16 — gathered+transposed fp16 blocks.
    XP: [128, tot8, 2, 128] e4m3 — pair-interleaved transposed blocks.
    mask_vals: optional [mb_all, kb] array; when given, each block is
    multiplied by its (non-unit) mask value before casting."""
    import ml_dtypes

    E4 = ml_dtypes.float8_e4m3
    P = BLOCK
    tot16 = int(sum(len(k) for k in kept16))
    tot8 = int(sum(len(p) for p in pairs))
    XC_np = np.empty((P, max(tot16, 1) * P), dtype=np.float16)
    XP_np = np.empty((P, max(tot8, 1), 2, P), dtype=E4)
    off16 = 0
    off8 = 0
    for si, row in enumerate(rows):
        ks = np.asarray(kept16[si], dtype=np.int64)
        if len(ks):
            blk = x4[row][:, ks, :]  # [m, c, k]
            t = np.ascontiguousarray(blk.transpose(2, 1, 0))  # [k, c, m]
            if mask_vals is not None:
                t = t * mask_vals[row][ks][None, :, None].astype(np.float32)
            XC_np[:, off16 * P : (off16 + len(ks)) * P] = (
                t.reshape(P, len(ks) * P).astype(np.float16)
            )
            off16 += len(ks)
        prs = pairs[si]
        if len(prs):
            pb = np.asarray(prs, dtype=np.int64).reshape(-1)  # [2p]
            blk = x4[row][:, pb, :]  # [m, 2p, k]
            t = blk.transpose(2, 1, 0)  # [k, 2p, m]
            if mask_vals is not None:
                t = t * mask_vals[row][pb][None, :, None].astype(np.float32)
            XP_np[:, off8 : off8 + len(prs)] = (
                np.ascontiguousarray(t).reshape(P, len(prs), 2, P).astype(E4)
            )
            off8 += len(prs)
    return XC_np, XP_np


def kernel(x, weight, block_mask):
    import jax
    import ml_dtypes
    from jax.sharding import NamedSharding, PartitionSpec

    E4 = ml_dtypes.float8_e4m3

    x = np.ascontiguousarray(x, dtype=np.float32)
    weight = np.ascontiguousarray(weight, dtype=np.float32)
    bm = np.asarray(block_mask)

    M, K = x.shape
    N = weight.shape[0]
    assert weight.shape == (N, K)
    mb, kb_blocks = bm.shape
    assert mb * BLOCK == M and kb_blocks * BLOCK == K
    P = BLOCK
    nsh = N // (N_CORES // N_GROUPS)  # per-core N shard (1024)

    all_kept = [np.flatnonzero(bm[s]) for s in range(mb)]
    mask_vals = None if set(np.unique(bm).tolist()) <= {0, 1} else bm
    all_counts = np.array([len(k) for k in all_kept], dtype=np.int64)
    scale = np.float32(1.0 / (1.0 - P_DROP))

    # balanced 2-way split of block-rows by kept count (greedy on sorted)
    order = np.argsort(-all_counts, kind="stable")
    group_rows = [[], []]
    sums = [0, 0]
    for r in order:
        g = 0 if sums[0] <= sums[1] else 1
        group_rows[g].append(int(r))
        sums[g] += int(all_counts[r])
    while abs(len(group_rows[0]) - len(group_rows[1])) > 0:
        big = 0 if len(group_rows[0]) > len(group_rows[1]) else 1
        group_rows[1 - big].append(group_rows[big].pop())

    # Greedy slot ordering per group: pick next the row introducing the
    # fewest new weight blocks (ties: smaller row), so the PE ramp only
    # waits for a small prefix of the weight shard.
    for g in (0, 1):
        remaining = set(group_rows[g])
        covered = set()
        ordered = []
        while remaining:
            best = min(
                remaining,
                key=lambda r: (len(set(map(int, all_kept[r])) - covered), r),
            )
            remaining.remove(best)
            ordered.append(best)
            covered |= set(map(int, all_kept[best]))
        group_rows[g] = ordered

    # Pair assignment, SPREAD evenly: every slot past the first
    # FP8_FREE_SLOTS gets ~FP8_FRAC of its kept blocks as fp8 pairs.
    # Concentrating fp8 DoubleRow work (2 MACs/cell/cycle) on all 8 cores
    # simultaneously trips the chip-level GPIO power brake (measured: a
    # periodic 81%-utilization duty cycle for the rest of the kernel), so
    # thin, even interleaving beats front-loading.
    pairs_all = [[] for _ in range(mb)]
    kept16_all = [list(map(int, k)) for k in all_kept]
    for g in (0, 1):
        rows = group_rows[g]
        budget = int(np.floor(FP8_FRAC * sum(all_counts[r] for r in rows) / 2.0 + 0.5))
        elig = rows[FP8_FREE_SLOTS:]
        quota = [int(np.floor(FP8_FRAC * all_counts[r] / 2.0 + 0.5)) for r in elig]
        # trim/extend quotas to the budget, round-robin
        total = sum(quota)
        i = 0
        while total > budget:
            if quota[i % len(elig)] > 0:
                quota[i % len(elig)] -= 1
                total -= 1
            i += 1
        i = 0
        while total < budget and i < 10 * len(elig):
            r = elig[i % len(elig)]
            if quota[i % len(elig)] < all_counts[r] // 2:
                quota[i % len(elig)] += 1
                total += 1
            i += 1
        for r, p in zip(elig, quota):
            if p > 0:
                k16 = kept16_all[r]
                c = len(k16)
                tail = k16[c - 2 * p :]
                pairs_all[r] = [(tail[2 * i2], tail[2 * i2 + 1]) for i2 in range(p)]
                kept16_all[r] = k16[: c - 2 * p]

    x4 = x.reshape(mb, P, kb_blocks, P)  # [row, m, b, k]
    wT = np.ascontiguousarray(weight.T) * (scale * np.float32(W_SCALE))  # [K, N]
    w4 = wT.reshape(kb_blocks, P, N)
    ws_quarters = [
        np.ascontiguousarray(w4[:, :, c * nsh : (c + 1) * nsh]).astype(np.float16)
        for c in range(CORES_PER_GROUP)
    ]
    w8_quarters = [
        np.ascontiguousarray(w4[:, :, c * nsh : (c + 1) * nsh]).astype(E4)
        for c in range(CORES_PER_GROUP)
    ]

    devices = jax.devices()
    assert len(devices) >= N_CORES

    group_data = []
    for g in (0, 1):
        rows = group_rows[g]
        kept16 = [kept16_all[r] for r in rows]
        prs = [pairs_all[r] for r in rows]
        XC_np, XP_np = _host_prep_group(x4, rows, kept16, prs, mask_vals=mask_vals)
        nc = _build_program(kept16, prs, nsh, kb_blocks)
        fn, in_names, out_names, out_avals, mesh = _make_fn(
            nc, devices[g * CORES_PER_GROUP : (g + 1) * CORES_PER_GROUP]
        )
        per_core = []
        for c in range(CORES_PER_GROUP):
            per_core.append(
                {"XC": XC_np, "XP": XP_np, "WS": ws_quarters[c], "W8": w8_quarters[c]}
            )
        concat_in = [
            np.concatenate([per_core[c][nm] for c in range(CORES_PER_GROUP)], axis=0)
            for nm in in_names
        ]
        sharding = NamedSharding(mesh, PartitionSpec("core"))
        dev_in = [jax.device_put(a, sharding) for a in concat_in]

        def zeros(out_avals=out_avals):
            return [
                np.zeros((CORES_PER_GROUP * av.shape[0], *av.shape[1:]), av.dtype)
                for av in out_avals
            ]

        group_data.append(
            dict(
                rows=rows,
                nc=nc,
                fn=fn,
                in_names=in_names,
                out_names=out_names,
                out_avals=out_avals,
                dev_in=dev_in,
                zeros=zeros,
                mesh=mesh,
            )
        )

    # --- execute (concurrent dispatch; first call also compiles) ---
    handles = []
    for gd in group_data:
        handles.append(gd["fn"](*gd["dev_in"], *gd["zeros"]()))
    jax.block_until_ready(handles)
    # materialize to host BEFORE any re-execution: donation can recycle the
    # first run's output buffers once another execution is dispatched
    host_outs = [
        [np.asarray(a) for a in handles[g]] for g in range(len(group_data))
    ]

    # --- optional profiled re-run (KERNEL_TRACE=1) ---
    LAST_RUN_INFO.clear()
    if os.environ.get("KERNEL_TRACE", "0") == "1":
        try:
            _profiled_rerun(group_data)
        except Exception as e:
            import traceback

            traceback.print_exc()
            print(f"kernel3: profiling failed ({e})", file=sys.stderr)

    # --- assemble ---
    out = np.empty((M, N), dtype=np.float32)
    for g, gd in enumerate(group_data):
        arrs = host_outs[g]
        mbg = len(gd["rows"])
        for i, nm in enumerate(gd["out_names"]):
            a = arrs[i].reshape(
                CORES_PER_GROUP, P, mbg, nsh
            )  # [core, m, slot, n]
            for c in range(CORES_PER_GROUP):
                t = a[c].transpose(1, 0, 2)  # [slot, m, n]
                for si, row in enumerate(gd["rows"]):
                    out[row * P : (row + 1) * P, c * nsh : (c + 1) * nsh] = t[si]
    return out


def _install_ntff_shim():
    """Provide antenv.axon_hooks with the ctypes NTFF profile hook."""
    import contextlib
    import ctypes
    import types

    so_path = "/opt/axon/libaxon_pjrt.so"

    try:
        from antenv.axon_hooks import get_axon_ntff_profile_hook  # noqa: F401

        return
    except ImportError:
        pass

    lib = ctypes.CDLL(so_path)
    if not hasattr(lib, "axon_start_nrt_profile"):
        raise RuntimeError("no axon_start_nrt_profile in libaxon_pjrt.so")
    lib.axon_start_nrt_profile.argtypes = [
        ctypes.POINTER(ctypes.c_int64),
        ctypes.c_size_t,
    ]
    lib.axon_start_nrt_profile.restype = ctypes.c_int64
    lib.axon_stop_nrt_profile.argtypes = [ctypes.c_char_p]
    lib.axon_stop_nrt_profile.restype = ctypes.c_int64

    @contextlib.contextmanager
    def _ctx(output_dir, device_ids):
        import jax

        jax.devices()
        if device_ids:
            ids = (ctypes.c_int64 * len(device_ids))(*device_ids)
            rc = lib.axon_start_nrt_profile(ids, len(device_ids))
        else:
            rc = lib.axon_start_nrt_profile(None, 0)
        if rc != 0:
            raise RuntimeError(f"axon_start_nrt_profile rc={rc}")
        try:
            yield
        finally:
            n = lib.axon_stop_nrt_profile(str(output_dir).encode())
            if n < 0:
                raise RuntimeError(f"axon_stop_nrt_profile rc={n}")
            print(f"profile: {n} file(s) written to {output_dir}")

    hook = _ctx

    def set_axon_ntff_profile_hook(h):
        pass

    def get_axon_ntff_profile_hook():
        return hook

    try:
        import antenv

        antenv_mod = antenv
    except ImportError:
        antenv_mod = types.ModuleType("antenv")
        antenv_mod.__path__ = []
        sys.modules["antenv"] = antenv_mod
    mod = types.ModuleType("antenv.axon_hooks")
    mod.set_axon_ntff_profile_hook = set_axon_ntff_profile_hook
    mod.get_axon_ntff_profile_hook = get_axon_ntff_profile_hook
    sys.modules["antenv.axon_hooks"] = mod
    antenv_mod.axon_hooks = mod


def _profiled_rerun(group_data):
    """Concurrent re-execution under the axon NTFF hook; fills LAST_RUN_INFO."""
    import glob
    import tempfile

    import jax

    _install_ntff_shim()

    from antenv.axon_hooks import get_axon_ntff_profile_hook

    hook = get_axon_ntff_profile_hook()
    neff_dir = tempfile.mkdtemp(prefix="k3prof_")
    trace_core = int(os.environ.get("KERNEL_TRACE_CORE", "0"))
    with hook(neff_dir, [trace_core]):
        handles = []
        for gd in group_data:
            handles.append(gd["fn"](*gd["dev_in"], *gd["zeros"]()))
        jax.block_until_ready(handles)

    ntffs = sorted(glob.glob(os.path.join(neff_dir, "*_body*.ntff")))
    if not ntffs:
        print(f"kernel3: no ntff produced in {neff_dir}", file=sys.stderr)
        return

    import re
    import shutil

    import gauge.profiler
    from concourse._compat import FishPath
    from concourse.bass_utils import _process_ntff_profile

    # One NTFF per executable (each group's shard_map numbers its devices
    # from 0, so both land as device000000). Executable ids are assigned at
    # compile time in group dispatch order: ascending id == group order.
    by_exec = {}
    for f in ntffs:
        m = re.search(r"executable(\d+)", os.path.basename(f))
        if m:
            by_exec.setdefault(int(m.group(1)), []).append(f)

    times = []
    infos = []
    for gi, execid in enumerate(sorted(by_exec)):
        if gi >= len(group_data):
            break
        nc = group_data[gi]["nc"]
        sub = os.path.join(neff_dir, f"exec{execid}")
        os.makedirs(sub, exist_ok=True)
        for f in glob.glob(os.path.join(neff_dir, f"*executable{execid:06d}*")):
            if os.path.isfile(f):
                shutil.move(f, os.path.join(sub, os.path.basename(f)))
        try:
            profile = gauge.profiler.Profile(
                profile_path=FishPath(sub),
                kernel_dev_mode=True,
                profile_on_exit=False,
                bass_kernel=nc.m,
                offline_processing=True,
                fname="*_body*",
                metadata={"artifacts_path": sub},
            )
            perf = _process_ntff_profile(
                profile,
                sub,
                nc,
                core_ids=[0],
                trace_cores=[0],
                stitch_traces=False,
                trace_kwargs={},
                trace_events=False,
            )
        except Exception as e:
            print(f"kernel3: profile of exec{execid} failed: {e}", file=sys.stderr)
            continue
        if perf.exec_time_ns is not None:
            times.append(perf.exec_time_ns)
        infos.append(
            dict(
                group=gi,
                exec_time_ns=perf.exec_time_ns,
                trace=perf.insts_and_trace_path[1]
                if perf.insts_and_trace_path
                else None,
                profile_json=perf.profile_json,
            )
        )
    LAST_RUN_INFO.update(
        exec_time_ns=max(times) if times else None,
        per_group=infos,
        trace=infos[0].get("trace") if infos else None,
        profile_json=infos[0].get("profile_json") if infos else None,
    )


# revision 27
# speedup vs baseline: 1.1019x; 1.1019x over previous
"""Blockwise-dropout GEMM (DropoutMM) for 8x Trainium2 NeuronCores — v3.

out = (x * expand(block_mask) / (1-p)) @ weight.T
  x: [8192, 4096] f32, weight: [4096, 4096] f32, block_mask: [64, 32] i32

v3 strategy (on top of v2's 2-group M-split x 4-way N-split, trace-time
block skipping, fp16 matmuls). Measured ~212-215us vs v2's ~245us; rel err
1.956e-2 (deterministic, FP8_FRAC=0.375) vs the 2e-2 gate:
  - Mixed-precision schedule: FP8_FRAC of each row's kept k-blocks is
    computed as fp8e4 (e4m3) DoubleRow PAIRS — two k-blocks interleaved per
    matmul at 2x PE rate — the rest stays fp16 (fp8-everything would be
    ~3.2e-2 err; fp16-only is 2.5e-4; err scales as 3.2e-2*sqrt(frac)).
    Pairs sit at each slot's tail, spread evenly across slots:
    concentrating DoubleRow work on all 8 cores at once trips a chip-level
    GPIO power brake (~0.81 util duty cycle for the rest of the kernel).
  - Both W copies are pre-scaled by 64 (w8 = e4m3(w*scale*64) to clear
    e4m3's subnormal floor at sigma~=1/64; w16 = fp16(w*scale*64) so both
    precisions accumulate into ONE PSUM bank), and the evacuation applies
    x 1/64 — no extra combine ops.
  - Weight preload (both precisions) on the scalar HWDGE ring as ONE
    combined first-use-ordered stream, full 2KB-per-partition-line blocks
    (the gpsimd ring is a slow software-DMA path; half-blocks halve DMA
    line size and cost ~27us). First FP8_FREE_SLOTS slots stay pure-fp16.
  - N_WARM dep-free warmup matmuls keep the PE busy through the ~15us
    p-state ramp window (idle gaps reset the ramp) while weights stream.
  - Evacuation split DVE(nt0)/ACT(nt1); stores split across the ACT and SP
    rings per half; the last slot evacuates in quarters to shrink the
    final drain.
"""

import os
import sys

import numpy as np

for _p in ("/opt/trn_rl_repo", "/root/.axon_site/_ro/trn_rl_repo"):
    if os.path.isdir(_p) and _p not in sys.path:
        sys.path.insert(0, _p)

BLOCK = 128
P_DROP = 0.1
N_CORES = 8
N_GROUPS = 2
CORES_PER_GROUP = N_CORES // N_GROUPS
MM_FREE = 512  # matmul moving free dim (one PSUM bank of fp32)
W_SCALE = 64.0  # fp8 weight pre-scale (power of 2; undone at evacuation)
FP8_FRAC = float(os.environ.get("KERNEL_FP8_FRAC", "0.375"))
FP8_FREE_SLOTS = int(os.environ.get("KERNEL_FREE_SLOTS", "2"))  # first slots per group stay pure fp16 (PE ramp)
N_WARM = int(os.environ.get("KERNEL_N_WARM", "28"))

LAST_RUN_INFO = {}


def _build_program(kept16, pairs, nsh, kb_blocks):
    """One group's program: exact block-skip GEMM over len(kept16) slots.

    kept16[s]: fp16 block indices for slot s; pairs[s]: list of (b0, b1)
    fp8 DoubleRow pairs (b0 < b1). Accumulation is 64x the true output;
    evacuation multiplies by 1/64.
    """
    from concourse import bacc
    import concourse.mybir as mybir
    import concourse.tile as tile

    P = BLOCK
    mb = len(kept16)
    c16 = [len(k) for k in kept16]
    np8 = [len(p) for p in pairs]
    tot16 = int(sum(c16))
    tot8 = int(sum(np8))
    cmax16 = max(1, max(c16))
    pmax = max(1, max(np8))
    nt_tiles = nsh // MM_FREE

    nc = bacc.Bacc("TRN2", target_bir_lowering=False)
    XC = nc.dram_tensor("XC", [P, max(tot16, 1) * P], mybir.dt.float16,
                        kind="ExternalInput")
    XP = nc.dram_tensor("XP", [P, max(tot8, 1), 2, P], mybir.dt.float8e4,
                        kind="ExternalInput")
    WS = nc.dram_tensor("WS", [kb_blocks, P, nsh], mybir.dt.float16,
                        kind="ExternalInput")
    W8 = nc.dram_tensor("W8", [kb_blocks, P, nsh], mybir.dt.float8e4,
                        kind="ExternalInput")
    OUT = nc.dram_tensor("OUT", [P, mb, nsh], mybir.dt.float32,
                         kind="ExternalOutput")

    # Preload order: combined first-use list over the slot program across
    # both precisions; all emitted on the scalar HWDGE ring so delivery
    # tracks need-order.
    w_order = []  # (is_fp8, block)
    seen16, seen8 = set(), set()
    for s in range(mb):
        for b0, b1 in pairs[s]:
            for b in (int(b0), int(b1)):
                if b not in seen8:
                    seen8.add(b)
                    w_order.append((True, b))
        for b in kept16[s]:
            if int(b) not in seen16:
                seen16.add(int(b))
                w_order.append((False, int(b)))

    with tile.TileContext(nc) as tc:
        with (
            tc.tile_pool(name="wpool", bufs=1) as wpool,
            tc.tile_pool(name="xpool", bufs=4) as xpool,
            tc.tile_pool(name="opool", bufs=3) as opool,
            tc.tile_pool(name="psum", bufs=8, space="PSUM") as psum,
        ):
            # HAM warmup: dummy matmuls with no DMA deps run during the fixed
            # kernel preamble so the PE clock is ramped when real work starts.
            wa = xpool.tile([P, P], mybir.dt.bfloat16, tag="warm_a", bufs=1)
            wb = xpool.tile([P, MM_FREE], mybir.dt.bfloat16, tag="warm_b", bufs=1)
            nc.gpsimd.memset(wa, 0.0)
            nc.gpsimd.memset(wb, 0.0)
            wp = psum.tile([P, MM_FREE], mybir.dt.float32, tag="ps", name="warm_ps")
            for _ in range(N_WARM):
                nc.tensor.matmul(wp, wa, wb, start=True, stop=True)

            # All weight preloads on the scalar HWDGE ring in need order —
            # the gpsimd ring is a SOFTWARE dma path (slow descriptor
            # generation) and starves the PE.
            w_res = wpool.tile([P, kb_blocks, nsh], mybir.dt.float16, tag="w")
            w8_res = wpool.tile([P, kb_blocks, nsh], mybir.dt.float8e4, tag="w8")
            for is8, b in w_order:
                if is8:
                    nc.scalar.dma_start(out=w8_res[:, b], in_=W8[b])
                else:
                    nc.scalar.dma_start(out=w_res[:, b], in_=WS[b])

            off16 = 0
            off8 = 0
            for s in range(mb):
                c = c16[s]
                p = np8[s]
                ot = opool.tile([P, nsh], mybir.dt.float32, tag="o")
                if c == 0 and p == 0:
                    nc.any.memset(ot, 0.0)
                    nc.scalar.dma_start(out=OUT[:, s, :], in_=ot[:])
                    continue
                if c > 0:
                    xt = xpool.tile(
                        [P, cmax16 * P], mybir.dt.float16, tag="x", name=f"x_{s}"
                    )
                    nc.sync.dma_start(
                        out=xt[:, : c * P], in_=XC[:, off16 * P : (off16 + c) * P]
                    )
                if p > 0:
                    xt8 = xpool.tile(
                        [P, pmax, 2, P], mybir.dt.float8e4, tag="x8", name=f"x8_{s}"
                    )
                    nc.sync.dma_start(
                        out=xt8[:, :p], in_=XP[:, off8 : off8 + p]
                    )
                pts = [
                    psum.tile(
                        [P, MM_FREE], mybir.dt.float32, tag="ps", name=f"ps_{s}_{nt}"
                    )
                    for nt in range(nt_tiles)
                ]
                units = p + c
                u = 0
                # fp8 pairs first, then fp16 singles; j-outer / nt-inner so
                # consecutive matmuls share the stationary operand.
                for j in range(p):
                    b0, b1 = int(pairs[s][j][0]), int(pairs[s][j][1])
                    st = b1 - b0
                    for nt in range(nt_tiles):
                        nc.tensor.matmul(
                            pts[nt],
                            xt8[:, j],
                            w8_res[:, b0 : b1 + 1 : st,
                                   nt * MM_FREE : (nt + 1) * MM_FREE],
                            start=(u == 0),
                            stop=(u == units - 1),
                            perf_mode=mybir.MatmulPerfMode.DoubleRow,
                        )
                    u += 1
                for j in range(c):
                    b = int(kept16[s][j])
                    for nt in range(nt_tiles):
                        nc.tensor.matmul(
                            pts[nt],
                            xt[:, j * P : (j + 1) * P],
                            w_res[:, b, nt * MM_FREE : (nt + 1) * MM_FREE],
                            start=(u == 0),
                            stop=(u == units - 1),
                        )
                    u += 1
                # Evacuation split across DVE (nt0) and ACT (nt1); stores
                # split per half across the ACT and SP rings — halves the
                # per-slot evacuation latency and the end-of-kernel flush.
                # The LAST slot goes in quarters for a shorter final drain.
                if s == mb - 1:
                    Q = MM_FREE // 2
                    for q in range(4):
                        sl = slice(q * Q, (q + 1) * Q)
                        psl = slice((q % 2) * Q, (q % 2 + 1) * Q)
                        eng = nc.vector if q % 2 == 0 else None
                        if eng is not None:
                            eng.tensor_scalar_mul(
                                out=ot[:, sl], in0=pts[q // 2][:, psl],
                                scalar1=1.0 / W_SCALE,
                            )
                        else:
                            nc.scalar.activation(
                                out=ot[:, sl],
                                in_=pts[q // 2][:, psl],
                                func=mybir.ActivationFunctionType.Copy,
                                scale=1.0 / W_SCALE,
                            )
                        ring = nc.scalar if q % 2 == 0 else nc.sync
                        ring.dma_start(out=OUT[:, s, sl], in_=ot[:, sl])
                else:
                    nc.vector.tensor_scalar_mul(
                        out=ot[:, 0:MM_FREE], in0=pts[0], scalar1=1.0 / W_SCALE
                    )
                    nc.scalar.activation(
                        out=ot[:, MM_FREE : 2 * MM_FREE],
                        in_=pts[1],
                        func=mybir.ActivationFunctionType.Copy,
                        scale=1.0 / W_SCALE,
                    )
                    nc.scalar.dma_start(
                        out=OUT[:, s, 0:MM_FREE], in_=ot[:, 0:MM_FREE]
                    )
                    nc.sync.dma_start(
                        out=OUT[:, s, MM_FREE : 2 * MM_FREE],
                        in_=ot[:, MM_FREE : 2 * MM_FREE],
                    )
                off16 += c
                off8 += p
    nc.compile()
    return nc


def _make_fn(nc, devices):
    """Replicates bass2jax.run_bass_via_pjrt's multi-core path for an
    arbitrary device subset; returns an async-dispatchable jitted fn."""
    import jax
    import concourse.mybir as mybir
    from concourse.bass2jax import (
        _bass_exec_p,
        install_neuronx_cc_hook,
        partition_id_tensor,
    )
    from jax.experimental.shard_map import shard_map
    from jax.sharding import Mesh, PartitionSpec

    install_neuronx_cc_hook()

    partition_name = nc.partition_id_tensor.name if nc.partition_id_tensor else None
    in_names, out_names, out_avals = [], [], []
    for alloc in nc.m.functions[0].allocations:
        if not isinstance(alloc, mybir.MemoryLocationSet):
            continue
        name = alloc.memorylocations[0].name
        if alloc.kind == "ExternalInput":
            if name != partition_name:
                in_names.append(name)
        elif alloc.kind == "ExternalOutput":
            shape = tuple(alloc.tensor_shape)
            dtype = mybir.dt.np(alloc.dtype)
            out_names.append(name)
            out_avals.append(jax.core.ShapedArray(shape, dtype))
    n_params = len(in_names)
    all_names = list(in_names) + list(out_names)
    if partition_name is not None:
        all_names.append(partition_name)

    def _body(*args):
        operands = list(args)
        if partition_name is not None:
            operands.append(partition_id_tensor())
        outs = _bass_exec_p.bind(
            *operands,
            out_avals=tuple(out_avals),
            in_names=tuple(all_names),
            out_names=tuple(out_names),
            lowering_input_output_aliases=(),
            sim_require_finite=True,
            sim_require_nnan=True,
            nc=nc,
        )
        return tuple(outs)

    mesh = Mesh(np.asarray(devices), ("core",))
    n_outs = len(out_names)
    donate = tuple(range(n_params, n_params + n_outs))
    fn = jax.jit(
        shard_map(
            _body,
            mesh=mesh,
            in_specs=(PartitionSpec("core"),) * (n_params + n_outs),
            out_specs=(PartitionSpec("core"),) * n_outs,
            check_rep=False,
        ),
        donate_argnums=donate,
        keep_unused=True,
    )
    return fn, in_names, out_names, out_avals, mesh


def _host_prep_group(x4, rows, kept16, pairs, mask_vals=None):
    """XC (fp16 singles) and XP (fp8 pairs) for one group.

    XC: [128, tot16*128] fp16 — gathered+transposed fp16 blocks.
    XP: [128, tot8, 2, 128] e4m3 — pair-interleaved transposed blocks.
    mask_vals: optional [mb_all, kb] array; when given, each block is
    multiplied by its (non-unit) mask value before casting."""
    import ml_dtypes

    E4 = ml_dtypes.float8_e4m3
    P = BLOCK
    tot16 = int(sum(len(k) for k in kept16))
    tot8 = int(sum(len(p) for p in pairs))
    XC_np = np.empty((P, max(tot16, 1) * P), dtype=np.float16)
    XP_np = np.empty((P, max(tot8, 1), 2, P), dtype=E4)
    off16 = 0
    off8 = 0
    for si, row in enumerate(rows):
        ks = np.asarray(kept16[si], dtype=np.int64)
        if len(ks):
            blk = x4[row][:, ks, :]  # [m, c, k]
            t = np.ascontiguousarray(blk.transpose(2, 1, 0))  # [k, c, m]
            if mask_vals is not None:
                t = t * mask_vals[row][ks][None, :, None].astype(np.float32)
            XC_np[:, off16 * P : (off16 + len(ks)) * P] = (
                t.reshape(P, len(ks) * P).astype(np.float16)
            )
            off16 += len(ks)
        prs = pairs[si]
        if len(prs):
            pb = np.asarray(prs, dtype=np.int64).reshape(-1)  # [2p]
            blk = x4[row][:, pb, :]  # [m, 2p, k]
            t = blk.transpose(2, 1, 0)  # [k, 2p, m]
            if mask_vals is not None:
                t = t * mask_vals[row][pb][None, :, None].astype(np.float32)
            XP_np[:, off8 : off8 + len(prs)] = (
                np.ascontiguousarray(t).reshape(P, len(prs), 2, P).astype(E4)
            )
            off8 += len(prs)
    return XC_np, XP_np


def kernel(x, weight, block_mask):
    import jax
    import ml_dtypes
    from jax.sharding import NamedSharding, PartitionSpec

    E4 = ml_dtypes.float8_e4m3

    x = np.ascontiguousarray(x, dtype=np.float32)
    weight = np.ascontiguousarray(weight, dtype=np.float32)
    bm = np.asarray(block_mask)

    M, K = x.shape
    N = weight.shape[0]
    assert weight.shape == (N, K)
    mb, kb_blocks = bm.shape
    assert mb * BLOCK == M and kb_blocks * BLOCK == K
    P = BLOCK
    nsh = N // (N_CORES // N_GROUPS)  # per-core N shard (1024)

    all_kept = [np.flatnonzero(bm[s]) for s in range(mb)]
    mask_vals = None if set(np.unique(bm).tolist()) <= {0, 1} else bm
    all_counts = np.array([len(k) for k in all_kept], dtype=np.int64)
    scale = np.float32(1.0 / (1.0 - P_DROP))

    # balanced 2-way split of block-rows by kept count (greedy on sorted)
    order = np.argsort(-all_counts, kind="stable")
    group_rows = [[], []]
    sums = [0, 0]
    for r in order:
        g = 0 if sums[0] <= sums[1] else 1
        group_rows[g].append(int(r))
        sums[g] += int(all_counts[r])
    while abs(len(group_rows[0]) - len(group_rows[1])) > 0:
        big = 0 if len(group_rows[0]) > len(group_rows[1]) else 1
        group_rows[1 - big].append(group_rows[big].pop())

    # Greedy slot ordering per group: pick next the row introducing the
    # fewest new weight blocks (ties: smaller row), so the PE ramp only
    # waits for a small prefix of the weight shard.
    for g in (0, 1):
        remaining = set(group_rows[g])
        covered = set()
        ordered = []
        while remaining:
            best = min(
                remaining,
                key=lambda r: (len(set(map(int, all_kept[r])) - covered), r),
            )
            remaining.remove(best)
            ordered.append(best)
            covered |= set(map(int, all_kept[best]))
        group_rows[g] = ordered

    # Pair assignment, SPREAD evenly: every slot past the first
    # FP8_FREE_SLOTS gets ~FP8_FRAC of its kept blocks as fp8 pairs.
    # Concentrating fp8 DoubleRow work (2 MACs/cell/cycle) on all 8 cores
    # simultaneously trips the chip-level GPIO power brake (measured: a
    # periodic 81%-utilization duty cycle for the rest of the kernel), so
    # thin, even interleaving beats front-loading.
    pairs_all = [[] for _ in range(mb)]
    kept16_all = [list(map(int, k)) for k in all_kept]
    for g in (0, 1):
        rows = group_rows[g]
        budget = int(np.floor(FP8_FRAC * sum(all_counts[r] for r in rows) / 2.0 + 0.5))
        elig = rows[FP8_FREE_SLOTS:]
        quota = [int(np.floor(FP8_FRAC * all_counts[r] / 2.0 + 0.5)) for r in elig]
        # trim/extend quotas to the budget, round-robin
        total = sum(quota)
        i = 0
        while total > budget:
            if quota[i % len(elig)] > 0:
                quota[i % len(elig)] -= 1
                total -= 1
            i += 1
        i = 0
        while total < budget and i < 10 * len(elig):
            r = elig[i % len(elig)]
            if quota[i % len(elig)] < all_counts[r] // 2:
                quota[i % len(elig)] += 1
                total += 1
            i += 1
        for r, p in zip(elig, quota):
            if p > 0:
                k16 = kept16_all[r]
                c = len(k16)
                tail = k16[c - 2 * p :]
                pairs_all[r] = [(tail[2 * i2], tail[2 * i2 + 1]) for i2 in range(p)]
                kept16_all[r] = k16[: c - 2 * p]

    x4 = x.reshape(mb, P, kb_blocks, P)  # [row, m, b, k]
    wT = np.ascontiguousarray(weight.T) * (scale * np.float32(W_SCALE))  # [K, N]
    w4 = wT.reshape(kb_blocks, P, N)
    ws_quarters = [
        np.ascontiguousarray(w4[:, :, c * nsh : (c + 1) * nsh]).astype(np.float16)
        for c in range(CORES_PER_GROUP)
    ]
    w8_quarters = [
        np.ascontiguousarray(w4[:, :, c * nsh : (c + 1) * nsh]).astype(E4)
        for c in range(CORES_PER_GROUP)
    ]

    devices = jax.devices()
    assert len(devices) >= N_CORES

    group_data = []
    for g in (0, 1):
        rows = group_rows[g]
        kept16 = [kept16_all[r] for r in rows]
        prs = [pairs_all[r] for r in rows]
        XC_np, XP_np = _host_prep_group(x4, rows, kept16, prs, mask_vals=mask_vals)
        nc = _build_program(kept16, prs, nsh, kb_blocks)
        fn, in_names, out_names, out_avals, mesh = _make_fn(
            nc, devices[g * CORES_PER_GROUP : (g + 1) * CORES_PER_GROUP]
        )
        per_core = []
        for c in range(CORES_PER_GROUP):
            per_core.append(
                {"XC": XC_np, "XP": XP_np, "WS": ws_quarters[c], "W8": w8_quarters[c]}
            )
        concat_in = [
            np.concatenate([per_core[c][nm] for c in range(CORES_PER_GROUP)], axis=0)
            for nm in in_names
        ]
        sharding = NamedSharding(mesh, PartitionSpec("core"))
        dev_in = [jax.device_put(a, sharding) for a in concat_in]

        def zeros(out_avals=out_avals):
            return [
                np.zeros((CORES_PER_GROUP * av.shape[0], *av.shape[1:]), av.dtype)
                for av in out_avals
            ]

        group_data.append(
            dict(
                rows=rows,
                nc=nc,
                fn=fn,
                in_names=in_names,
                out_names=out_names,
                out_avals=out_avals,
                dev_in=dev_in,
                zeros=zeros,
                mesh=mesh,
            )
        )

    # --- execute (concurrent dispatch; first call also compiles) ---
    handles = []
    for gd in group_data:
        handles.append(gd["fn"](*gd["dev_in"], *gd["zeros"]()))
    jax.block_until_ready(handles)
    # materialize to host BEFORE any re-execution: donation can recycle the
    # first run's output buffers once another execution is dispatched
    host_outs = [
        [np.asarray(a) for a in handles[g]] for g in range(len(group_data))
    ]

    # --- optional profiled re-run (KERNEL_TRACE=1) ---
    LAST_RUN_INFO.clear()
    if os.environ.get("KERNEL_TRACE", "0") == "1":
        try:
            _profiled_rerun(group_data)
        except Exception as e:
            import traceback

            traceback.print_exc()
            print(f"kernel3: profiling failed ({e})", file=sys.stderr)

    # --- assemble ---
    out = np.empty((M, N), dtype=np.float32)
    for g, gd in enumerate(group_data):
        arrs = host_outs[g]
        mbg = len(gd["rows"])
        for i, nm in enumerate(gd["out_names"]):
            a = arrs[i].reshape(
                CORES_PER_GROUP, P, mbg, nsh
            )  # [core, m, slot, n]
            for c in range(CORES_PER_GROUP):
                t = a[c].transpose(1, 0, 2)  # [slot, m, n]
                for si, row in enumerate(gd["rows"]):
                    out[row * P : (row + 1) * P, c * nsh : (c + 1) * nsh] = t[si]
    return out


def _install_ntff_shim():
    """Provide antenv.axon_hooks with the ctypes NTFF profile hook."""
    import contextlib
    import ctypes
    import types

    so_path = "/opt/axon/libaxon_pjrt.so"

    try:
        from antenv.axon_hooks import get_axon_ntff_profile_hook  # noqa: F401

        return
    except ImportError:
        pass

    lib = ctypes.CDLL(so_path)
    if not hasattr(lib, "axon_start_nrt_profile"):
        raise RuntimeError("no axon_start_nrt_profile in libaxon_pjrt.so")
    lib.axon_start_nrt_profile.argtypes = [
        ctypes.POINTER(ctypes.c_int64),
        ctypes.c_size_t,
    ]
    lib.axon_start_nrt_profile.restype = ctypes.c_int64
    lib.axon_stop_nrt_profile.argtypes = [ctypes.c_char_p]
    lib.axon_stop_nrt_profile.restype = ctypes.c_int64

    @contextlib.contextmanager
    def _ctx(output_dir, device_ids):
        import jax

        jax.devices()
        if device_ids:
            ids = (ctypes.c_int64 * len(device_ids))(*device_ids)
            rc = lib.axon_start_nrt_profile(ids, len(device_ids))
        else:
            rc = lib.axon_start_nrt_profile(None, 0)
        if rc != 0:
            raise RuntimeError(f"axon_start_nrt_profile rc={rc}")
        try:
            yield
        finally:
            n = lib.axon_stop_nrt_profile(str(output_dir).encode())
            if n < 0:
                raise RuntimeError(f"axon_stop_nrt_profile rc={n}")
            print(f"profile: {n} file(s) written to {output_dir}")

    hook = _ctx

    def set_axon_ntff_profile_hook(h):
        pass

    def get_axon_ntff_profile_hook():
        return hook

    try:
        import antenv

        antenv_mod = antenv
    except ImportError:
        antenv_mod = types.ModuleType("antenv")
        antenv_mod.__path__ = []
        sys.modules["antenv"] = antenv_mod
    mod = types.ModuleType("antenv.axon_hooks")
    mod.set_axon_ntff_profile_hook = set_axon_ntff_profile_hook
    mod.get_axon_ntff_profile_hook = get_axon_ntff_profile_hook
    sys.modules["antenv.axon_hooks"] = mod
    antenv_mod.axon_hooks = mod


def _profiled_rerun(group_data):
    """Concurrent re-execution under the axon NTFF hook; fills LAST_RUN_INFO."""
    import glob
    import tempfile

    import jax

    _install_ntff_shim()

    from antenv.axon_hooks import get_axon_ntff_profile_hook

    hook = get_axon_ntff_profile_hook()
    neff_dir = tempfile.mkdtemp(prefix="k3prof_")
    trace_core = int(os.environ.get("KERNEL_TRACE_CORE", "0"))
    with hook(neff_dir, [trace_core]):
        handles = []
        for gd in group_data:
            handles.append(gd["fn"](*gd["dev_in"], *gd["zeros"]()))
        jax.block_until_ready(handles)

    ntffs = sorted(glob.glob(os.path.join(neff_dir, "*_body*.ntff")))
    if not ntffs:
        print(f"kernel3: no ntff produced in {neff_dir}", file=sys.stderr)
        return

    import re
    import shutil

    import gauge.profiler
    from concourse._compat import FishPath
    from concourse.bass_utils import _process_ntff_profile

    # One NTFF per executable (each group's shard_map numbers its devices
    # from 0, so both land as device000000). Executable ids are assigned at
    # compile time in group dispatch order: ascending id == group order.
    by_exec = {}
    for f in ntffs:
        m = re.search(r"executable(\d+)", os.path.basename(f))
        if m:
            by_exec.setdefault(int(m.group(1)), []).append(f)

    times = []
    infos = []
    for gi, execid in enumerate(sorted(by_exec)):
        if gi >= len(group_data):
            break
        nc = group_data[gi]["nc"]
        sub = os.path.join(neff_dir, f"exec{execid}")
        os.makedirs(sub, exist_ok=True)
        for f in glob.glob(os.path.join(neff_dir, f"*executable{execid:06d}*")):
            if os.path.isfile(f):
                shutil.move(f, os.path.join(sub, os.path.basename(f)))
        try:
            profile = gauge.profiler.Profile(
                profile_path=FishPath(sub),
                kernel_dev_mode=True,
                profile_on_exit=False,
                bass_kernel=nc.m,
                offline_processing=True,
                fname="*_body*",
                metadata={"artifacts_path": sub},
            )
            perf = _process_ntff_profile(
                profile,
                sub,
                nc,
                core_ids=[0],
                trace_cores=[0],
                stitch_traces=False,
                trace_kwargs={},
                trace_events=False,
            )
        except Exception as e:
            print(f"kernel3: profile of exec{execid} failed: {e}", file=sys.stderr)
            continue
        if perf.exec_time_ns is not None:
            times.append(perf.exec_time_ns)
        infos.append(
            dict(
                group=gi,
                exec_time_ns=perf.exec_time_ns,
                trace=perf.insts_and_trace_path[1]
                if perf.insts_and_trace_path
                else None,
                profile_json=perf.profile_json,
            )
        )
    LAST_RUN_INFO.update(
        exec_time_ns=max(times) if times else None,
        per_group=infos,
        trace=infos[0].get("trace") if infos else None,
        profile_json=infos[0].get("profile_json") if infos else None,
    )
